# revision 59
# baseline (speedup 1.0000x reference)
"""Trainium2 Bass kernel for a GPT-style transformer block (B=2, T=2048, C=768, NH=12).

Sharding: 8 cores = 2 groups of 4 (one batch each); 512 tokens per core for
every phase. Collectives are three pipelined group-local AllGathers (k pairs
0-2, v, k pairs 3-5) sized so each hides under local compute; v is gathered in
fp8e4 (the local diagonal keeps a bf16 copy, which preserves the max-error
tail since concentrated softmax weights sit near the diagonal). Causality
across chunks is enforced by zero-scaling the gathered V (kills numerator and
denominator exactly); gathered chunk 3 is never needed by any core and is
skipped. Attention runs in two phases: D = all heads' local triangular
diagonals (no gather dependency, fills the collective window) into per-head
SBUF accumulators; G = gathered chunks, software-pipelined so PV matmuls of
group g-1 issue after the scores of group g (exp latency never stalls the
in-order PE queue), with the softmax denominator riding as a fused ones
column and normalized per head via a broadcast matmul + 64-lane reciprocal.

All matmuls use full-128-partition weight geometry (the PE runs ~40% slower
otherwise): scores contract over a 128-row k tile against zero-padded
per-half q; PV weights are read 128 wide out of 63-padded v tiles (output
rows 65-127 garbage, never read); attn_proj packs head pairs into [128,128]
weights. Weights and activations are bf16 (fp32 PSUM accumulate); LN1 stats
come from DVE bn_stats on the token-major x tiles; LN2 sums are fused into
the attn_proj loop; the MLP runs fc+gelu for all blocks, then per-output
block proj accumulation with residual/transpose/store overlapped.
"""

import contextlib

import numpy as np

import concourse.bacc as bacc
import concourse.bass as bass
import concourse.tile as tile
from concourse import mybir
from concourse.bass_utils import run_bass_kernel_spmd
from concourse.masks import make_identity

f32 = mybir.dt.float32
f32r = mybir.dt.float32r
bf16 = mybir.dt.bfloat16
f8 = mybir.dt.float8e4
AF = mybir.ActivationFunctionType
OP = mybir.AluOpType

B, T, C, NH, HD = 2, 2048, 768, 12, 64
EPS = 1e-5
N_CORES, GROUP = 8, 4
TOK = 512                      # tokens per core
KC = C // 128                  # 6 feature chunks
FF = 4 * C                     # 3072
MT = FF // 128                 # 24 ff chunks
VROW = HD + 1                  # v row with ones column
HROW = NH * VROW               # all-head v row: 780 elements per token
KBLK = NH * HD * TOK           # gathered k block per core
VBLK = TOK * HROW              # gathered v block per core
NCH = GROUP - 1                # gathered chunks actually used (chunk 3 unused)
SCALE = 1.0 / np.sqrt(HD)

_CACHE = {}


def _ap(handle, offset, pattern):
    return bass.AP(tensor=handle, offset=offset, ap=[list(p) for p in pattern])


def _build():
    nc = bacc.Bacc("TRN2", target_bir_lowering=False, debug=False,
                   num_devices=N_CORES)

    x_d = nc.dram_tensor("x", [TOK, C], f32, kind="ExternalInput")
    wqkv_d = nc.dram_tensor("wqkv", [C, 3 * C], bf16, kind="ExternalInput")
    bqkv_d = nc.dram_tensor("bqkv", [3 * C], f32, kind="ExternalInput")
    bqv_d = nc.dram_tensor("bqv", [C], bf16, kind="ExternalInput")
    apw_d = nc.dram_tensor("apw", [C, C], bf16, kind="ExternalInput")
    apb_d = nc.dram_tensor("apb", [C], f32, kind="ExternalInput")
    ln1s_d = nc.dram_tensor("ln1s", [C], bf16, kind="ExternalInput")
    ln1b_d = nc.dram_tensor("ln1b", [C], bf16, kind="ExternalInput")
    ln2s_d = nc.dram_tensor("ln2s", [C], bf16, kind="ExternalInput")
    ln2b_d = nc.dram_tensor("ln2b", [C], bf16, kind="ExternalInput")
    fcw_d = nc.dram_tensor("fcw", [C, FF], bf16, kind="ExternalInput")
    fcb_d = nc.dram_tensor("fcb", [FF], f32, kind="ExternalInput")
    pjw_d = nc.dram_tensor("pjw", [FF, C], bf16, kind="ExternalInput")
    pjb_d = nc.dram_tensor("pjb", [C], f32, kind="ExternalInput")
    zmask_d = nc.dram_tensor("zmask", [GROUP], f32, kind="ExternalInput")
    out_d = nc.dram_tensor("out", [TOK, C], f32, kind="ExternalOutput")

    KH = KBLK // 2
    kin1_d = nc.dram_tensor("k_in1", [KH], bf16, kind="Internal")
    kout1_d = nc.dram_tensor("k_out1", [GROUP, KH], bf16, kind="Internal")
    kin2_d = nc.dram_tensor("k_in2", [KH], bf16, kind="Internal")
    kout2_d = nc.dram_tensor("k_out2", [GROUP, KH], bf16, kind="Internal")
    vin_d = nc.dram_tensor("v_in", [VBLK], f8, kind="Internal")
    vout_d = nc.dram_tensor("v_out", [GROUP, VBLK], f8, kind="Internal")

    groups = [list(range(g * GROUP, (g + 1) * GROUP)) for g in range(2)]

    with tile.TileContext(nc) as tc:
        with (
            tc.tile_pool(name="const", bufs=1) as cst,
            tc.tile_pool(name="resid", bufs=1) as res_pool,
            tc.tile_pool(name="whoist", bufs=1) as wh_pool,
        ):
            # ---------------- constants ----------------
            ones_f = cst.tile([128, 128], f32)
            nc.vector.memset(ones_f[:], 1.0)
            ones_r = cst.tile([128, 128], f32r)
            nc.vector.tensor_copy(ones_r[:], ones_f[:])
            ones_b = cst.tile([128, 128], bf16)
            nc.vector.memset(ones_b[:], 1.0)
            ones_row_b = cst.tile([1, TOK], bf16)
            nc.vector.memset(ones_row_b[:], 1.0)
            ones12 = cst.tile([128, NH, 1], f32)
            nc.vector.memset(ones12[:], 1.0)
            eps_t = cst.tile([1, 1], f32)
            nc.vector.memset(eps_t[:], EPS)
            # causal 0/1 mask: keep where kv_p <= q_f
            m01_f = cst.tile([128, 128], f32)
            nc.vector.memset(m01_f[:], 1.0)
            nc.gpsimd.affine_select(
                out=m01_f[:], in_=m01_f[:], compare_op=OP.is_ge, fill=0.0,
                base=0, pattern=[[1, 128]], channel_multiplier=-1)
            m01 = cst.tile([128, 128], bf16)
            nc.vector.tensor_copy(m01[:], m01_f[:])
            ident = cst.tile([128, 128], f32)
            make_identity(nc, ident[:])

            sc1 = cst.tile([1, C], bf16)
            nc.sync.dma_start(sc1[:], ln1s_d.ap()[None, :])
            sb1 = cst.tile([1, C], bf16)
            nc.sync.dma_start(sb1[:], ln1b_d.ap()[None, :])
            sc2 = cst.tile([1, C], bf16)
            nc.sync.dma_start(sc2[:], ln2s_d.ap()[None, :])
            sb2 = cst.tile([1, C], bf16)
            nc.sync.dma_start(sb2[:], ln2b_d.ap()[None, :])
            bqk = cst.tile([128, 2 * C // 128], f32)
            nc.sync.dma_start(
                bqk[:], bqkv_d.ap()[0:2 * C].rearrange("(a p) -> p a", p=128))
            bv_row = cst.tile([1, C], bf16)
            nc.sync.dma_start(bv_row[:], bqv_d.ap()[None, :])
            apb = cst.tile([128, KC], f32)
            nc.sync.dma_start(
                apb[:], apb_d.ap().rearrange("(a p) -> p a", p=128))
            fcb = cst.tile([128, MT], f32)
            nc.sync.dma_start(
                fcb[:], fcb_d.ap().rearrange("(a p) -> p a", p=128))
            pjb = cst.tile([128, KC], f32)
            nc.sync.dma_start(
                pjb[:], pjb_d.ap().rearrange("(a p) -> p a", p=128))
            zm = cst.tile([128, GROUP], f32)
            nc.sync.dma_start(zm[:], _ap(zmask_d, 0, [[0, 128], [1, GROUP]]))

            # ---------------- residual stream xT + LN1 stats ----------
            # token-major load (contiguous lines), PE-transpose to
            # feature-major; LN1 mean/var come from DVE bn_stats on the
            # token-major tiles (off the PE critical path), transposed into
            # a [2, TOK] stats row pair.
            xT = [res_pool.tile([128, TOK], f32r, name=f"xT{k}")
                  for k in range(KC)]
            mu_row = res_pool.tile([1, TOK], f32, name="mu_row")
            var_row = res_pool.tile([1, TOK], f32, name="var_row")
            with (
                tc.tile_pool(name="xtm", bufs=2) as xtm_pool,
                tc.tile_pool(name="bns", bufs=2) as bn_pool,
                tc.tile_pool(name="tps", bufs=4, space="PSUM") as t_ps,
                tc.tile_pool(name="stps0", bufs=1, space="PSUM") as s_ps,
            ):
                mu_T = s_ps.tile([1, TOK], f32, name="mu_T")
                var_T = s_ps.tile([1, TOK], f32, name="var_T")
                for tt in range(4):
                    x_tm = xtm_pool.tile([128, C], f32, name="x_tm")
                    nc.sync.dma_start(
                        x_tm[:], x_d.ap()[tt * 128:(tt + 1) * 128, :])
                    bn6 = bn_pool.tile([128, 2, 6], f32, name="bn6")
                    nc.vector.bn_stats(bn6[:, 0, :], x_tm[:, 0:C // 2])
                    nc.vector.bn_stats(bn6[:, 1, :], x_tm[:, C // 2:C])
                    bnagg = bn_pool.tile([128, 2], f32, name="bnagg")
                    nc.vector.bn_aggr(bnagg[:], bn6[:])
                    csl = slice(tt * 128, (tt + 1) * 128)
                    nc.tensor.transpose(
                        mu_T[:, csl], bnagg[:, 0:1], ident[:])
                    nc.tensor.transpose(
                        var_T[:, csl], bnagg[:, 1:2], ident[:])
                    for k in range(KC):
                        tp = t_ps.tile([128, 128], f32, name="tp")
                        nc.tensor.transpose(
                            tp[:], x_tm[:, k * 128:(k + 1) * 128], ident[:])
                        nc.scalar.activation(
                            xT[k][:, tt * 128:(tt + 1) * 128], tp[:],
                            AF.Copy, bias=0.0, scale=1.0)
                nc.vector.tensor_copy(mu_row[:], mu_T[:])
                nc.vector.tensor_copy(var_row[:], var_T[:])

            # ---------------- LayerNorm helper (dst bf16) ----------------
            def layer_norm(src, sc_row, sb_row, dst_pool, tag):
                dst = [dst_pool.tile([128, TOK], bf16, name=f"h{tag}{k}")
                       for k in range(KC)]
                with (
                    tc.tile_pool(name=f"lnps{tag}", bufs=1, space="PSUM") as lps,
                    tc.tile_pool(name=f"lnab{tag}", bufs=2, space="PSUM") as aps,
                    tc.tile_pool(name=f"lnsb{tag}", bufs=3) as lsb,
                ):
                    sum_x = lps.tile([1, TOK], f32, name=f"sumx{tag}")
                    sum_sq = lps.tile([1, TOK], f32, name=f"sumsq{tag}")
                    for k in range(KC):
                        sq = lsb.tile([128, TOK], f32r, name=f"sq{tag}")
                        nc.scalar.activation(sq[:], src[k][:], AF.Square)
                        nc.tensor.matmul(sum_x[:], ones_r[:, 0:1], src[k][:],
                                         start=(k == 0), stop=(k == KC - 1))
                        nc.tensor.matmul(sum_sq[:], ones_r[:, 0:1], sq[:],
                                         start=(k == 0), stop=(k == KC - 1))
                    mu = lsb.tile([1, TOK], f32, name=f"mu{tag}")
                    nc.scalar.activation(mu[:], sum_x[:], AF.Copy,
                                         bias=0.0, scale=1.0 / C)
                    ex2 = lsb.tile([1, TOK], f32, name=f"ex2{tag}")
                    nc.scalar.activation(ex2[:], sum_sq[:], AF.Copy,
                                         bias=0.0, scale=1.0 / C)
                    var = lsb.tile([1, TOK], f32, name=f"var{tag}")
                    nc.vector.tensor_mul(var[:], mu[:], mu[:])
                    nc.vector.tensor_sub(var[:], ex2[:], var[:])
                    sd = lsb.tile([1, TOK], f32, name=f"sd{tag}")
                    nc.scalar.activation(sd[:], var[:], AF.Sqrt,
                                         bias=eps_t[:], scale=1.0)
                    rstd = lsb.tile([1, TOK], bf16, name=f"rstd{tag}")
                    with nc.allow_low_precision(reason="bf16 rstd for PE"):
                        nc.vector.reciprocal(rstd[:], sd[:])
                    nmr = lsb.tile([1, TOK], bf16, name=f"nmr{tag}")
                    nc.vector.scalar_tensor_tensor(
                        out=nmr[:], in0=mu[:], scalar=-1.0, in1=rstd[:],
                        op0=OP.mult, op1=OP.mult)
                    for k in range(KC):
                        a_ps = aps.tile([128, TOK], f32, name=f"aps{tag}")
                        b_ps = aps.tile([128, TOK], f32, name=f"bps{tag}")
                        sl = slice(k * 128, (k + 1) * 128)
                        nc.tensor.matmul(a_ps[:], sc_row[0:1, sl], rstd[:],
                                         start=True, stop=True)
                        nc.tensor.matmul(b_ps[:], sc_row[0:1, sl], nmr[:],
                                         start=True, stop=False)
                        nc.tensor.matmul(b_ps[:], sb_row[0:1, sl],
                                         ones_row_b[:], start=False, stop=True)
                        tmp = lsb.tile([128, TOK], f32, name=f"tmp{tag}")
                        nc.vector.tensor_mul(tmp[:], src[k][:], a_ps[:])
                        nc.vector.tensor_add(dst[k][:], tmp[:], b_ps[:])
                return dst

            # ---------------- LN1 + QKV projection ----------------
            qa_ctx = contextlib.ExitStack()
            qk_keep = qa_ctx.enter_context(tc.tile_pool(name="qkkeep", bufs=1))
            vt_pool = qa_ctx.enter_context(tc.tile_pool(name="vtp", bufs=1))
            ctx_pool = qa_ctx.enter_context(tc.tile_pool(name="ctxp", bufs=1))
            hln_ctx = contextlib.ExitStack()
            hln_pool = hln_ctx.enter_context(tc.tile_pool(name="hlnp", bufs=1))
            hln = [hln_pool.tile([128, TOK], bf16, name=f"h1{k}")
                   for k in range(KC)]
            with (
                tc.tile_pool(name="lnab1", bufs=2, space="PSUM") as aps1,
                tc.tile_pool(name="lnsb1", bufs=3) as lsb1,
            ):
                sd = lsb1.tile([1, TOK], f32, name="sd1")
                nc.scalar.activation(sd[:], var_row[:], AF.Sqrt,
                                     bias=eps_t[:], scale=1.0)
                rstd = lsb1.tile([1, TOK], bf16, name="rstd1")
                with nc.allow_low_precision(reason="bf16 rstd for PE"):
                    nc.vector.reciprocal(rstd[:], sd[:])
                nmr = lsb1.tile([1, TOK], bf16, name="nmr1")
                nc.vector.scalar_tensor_tensor(
                    out=nmr[:], in0=mu_row[:], scalar=-1.0, in1=rstd[:],
                    op0=OP.mult, op1=OP.mult)
                for k in range(KC):
                    a_ps = aps1.tile([128, TOK], f32, name="aps1")
                    b_ps = aps1.tile([128, TOK], f32, name="bps1")
                    sl = slice(k * 128, (k + 1) * 128)
                    nc.tensor.matmul(a_ps[:], sc1[0:1, sl], rstd[:],
                                     start=True, stop=True)
                    nc.tensor.matmul(b_ps[:], sc1[0:1, sl], nmr[:],
                                     start=True, stop=False)
                    nc.tensor.matmul(b_ps[:], sb1[0:1, sl],
                                     ones_row_b[:], start=False, stop=True)
                    tmp = lsb1.tile([128, TOK], f32, name="tmp1")
                    nc.vector.tensor_mul(tmp[:], xT[k][:], a_ps[:])
                    nc.vector.tensor_add(hln[k][:], tmp[:], b_ps[:])

            # qz[i][:, half, :]: rows of the OTHER half zeroed, so score
            # matmuls can use full-128-row weights (fast PE geometry)
            qz = [qk_keep.tile([128, 2, TOK], bf16, name=f"qz{i}")
                  for i in range(6)]
            k_pair = [qk_keep.tile([128, TOK], bf16, name=f"kp{i}")
                      for i in range(6)]
            # flat v tiles padded by 63 cols so PV weights can be read
            # 128 wide for any head (rows 65-127 of PV output unread)
            v_t = [vt_pool.tile([128, HROW + 63], bf16, name=f"vt{tt}")
                   for tt in range(4)]
            vt8 = [vt_pool.tile([128, NH, VROW], f8, name=f"vt8_{tt}")
                   for tt in range(4)]

            with (
                tc.tile_pool(name="wqkv", bufs=1) as wq_pool,
                tc.tile_pool(name="qkps", bufs=3, space="PSUM") as qk_ps,
                tc.tile_pool(name="vps", bufs=2, space="PSUM") as v_ps,
            ):
                wq = [wq_pool.tile([128, 3 * C], bf16, name=f"wq{k}")
                      for k in range(KC)]
                for k in range(KC):
                    nc.sync.dma_start(
                        wq[k][:],
                        _ap(wqkv_d, k * 128 * 3 * C, [[3 * C, 128], [1, 3 * C]]))
                # k pairs first (heads 2i,2i+1 at partitions 0/64)
                for i in range(6):
                    ps = qk_ps.tile([128, TOK], f32, name="qkp")
                    col = C + i * 128
                    for k in range(KC):
                        nc.tensor.matmul(
                            ps[:], wq[k][:, col:col + 128], hln[k][:],
                            start=(k == 0), stop=(k == KC - 1))
                    nc.scalar.activation(k_pair[i][:], ps[:], AF.Identity,
                                         bias=bqk[:, 6 + i:7 + i], scale=1.0)
                    kin = kin1_d if i < 3 else kin2_d
                    nc.sync.dma_start(
                        _ap(kin, (i % 3) * 128 * TOK, [[TOK, 128], [1, TOK]]),
                        k_pair[i][:])
                    if i == 2:
                        nc.gpsimd.collective_compute(
                            "AllGather", OP.bypass, replica_groups=groups,
                            ins=[kin1_d.ap().opt()],
                            outs=[kout1_d.ap().opt()])

                # v token-major, all heads + ones column, bias folded
                for tt in range(4):
                    vp = v_ps.tile([128, C], f32, name="vp")
                    nc.tensor.matmul(vp[:, 0:512], ones_b[0:1, :],
                                     bv_row[0:1, 0:512],
                                     start=True, stop=False)
                    nc.tensor.matmul(vp[:, 512:768], ones_b[0:1, :],
                                     bv_row[0:1, 512:768],
                                     start=True, stop=False)
                    tsl = slice(tt * 128, (tt + 1) * 128)
                    for k in range(KC):
                        nc.tensor.matmul(
                            vp[:, 0:512], hln[k][:, tsl],
                            wq[k][:, 2 * C:2 * C + 512],
                            start=False, stop=(k == KC - 1))
                        nc.tensor.matmul(
                            vp[:, 512:768], hln[k][:, tsl],
                            wq[k][:, 2 * C + 512:3 * C],
                            start=False, stop=(k == KC - 1))
                    vt3 = v_t[tt][:, 0:HROW].rearrange(
                        "p (h e) -> p h e", e=VROW)
                    nc.vector.tensor_copy(
                        vt3[:, :, 0:HD],
                        vp[:].rearrange("p (h e) -> p h e", e=HD))
                    nc.vector.tensor_copy(vt3[:, :, HD:VROW], ones12[:])
                    nc.vector.memset(v_t[tt][:, HROW:HROW + 63], 0.0)
                    nc.vector.tensor_copy(
                        vt8[tt][:, :, 0:HD],
                        vp[:].rearrange("p (h e) -> p h e", e=HD))
                    nc.vector.tensor_copy(vt8[tt][:, :, HD:VROW], ones12[:])
                    nc.sync.dma_start(
                        _ap(vin_d, tt * 128 * HROW,
                            [[HROW, 128], [1, HROW]]),
                        vt8[tt][:])
                nc.gpsimd.collective_compute(
                    "AllGather", OP.bypass, replica_groups=groups,
                    ins=[vin_d.ap().opt()], outs=[vout_d.ap().opt()])
                nc.gpsimd.collective_compute(
                    "AllGather", OP.bypass, replica_groups=groups,
                    ins=[kin2_d.ap().opt()], outs=[kout2_d.ap().opt()])
                # q pairs (stay local), split into zero-padded halves
                for i in range(6):
                    ps = qk_ps.tile([128, TOK], f32, name="qkp")
                    for k in range(KC):
                        nc.tensor.matmul(
                            ps[:], wq[k][:, i * 128:(i + 1) * 128], hln[k][:],
                            start=(k == 0), stop=(k == KC - 1))
                    nc.vector.memset(qz[i][64:128, 0, :], 0.0)
                    nc.vector.memset(qz[i][0:64, 1, :], 0.0)
                    nc.scalar.activation(qz[i][0:64, 0, :], ps[0:64, :],
                                         AF.Identity,
                                         bias=bqk[0:64, i:i + 1], scale=1.0)
                    nc.scalar.activation(qz[i][64:128, 1, :], ps[64:128, :],
                                         AF.Identity,
                                         bias=bqk[64:128, i:i + 1], scale=1.0)

            hln_ctx.close()

            # MLP/proj weights: allocate now, DMAs issued after the gather
            # reads below so they stream during attention without delaying
            # the critical kg/va loads
            apw = [wh_pool.tile([128, C], bf16, name=f"apw{p}")
                   for p in range(6)]
            fcw = [wh_pool.tile([128, FF], bf16, name=f"fcw{k}")
                   for k in range(KC)]

            # ---------------- attention (local q, gathered k/v) -------------
            # one PSUM accumulation chain per head over NCH gathered chunks
            # plus the local triangular diagonal; denominator rides in row 64.
            ctx_pair = [ctx_pool.tile([128, TOK], bf16, name=f"ctxp{i}")
                        for i in range(6)]
            acc_pool = qa_ctx.enter_context(tc.tile_pool(name="atacc", bufs=1))
            ctx_acc = [acc_pool.tile([VROW, TOK], bf16, name=f"cacc{h}")
                       for h in range(NH)]
            with (
                tc.tile_pool(name="atkg", bufs=2) as kg_pool,
                tc.tile_pool(name="atva", bufs=1) as va_pool,
                tc.tile_pool(name="ate", bufs=2) as e_pool,
                tc.tile_pool(name="nrm", bufs=3) as nrm_pool,
                tc.tile_pool(name="stps", bufs=2, space="PSUM") as st_ps,
                tc.tile_pool(name="pvps", bufs=2, space="PSUM") as pv_ps,
                tc.tile_pool(name="rbps", bufs=2, space="PSUM") as rb_ps,
            ):
                # gathered v for chunks 0..NCH-1, zero-scaled by causal chunk
                # mask (also zeroes the ones column => denominator exact)
                # kg loads for pairs 0-2 first: they depend only on the
                # k1 gather, so they must not queue behind the va DMAs
                # (which wait for the later v gather)
                kg_early = {}
                for i in range(3):
                    for c in range(NCH):
                        kgt = va_pool.tile([128, TOK], bf16,
                                           name=f"kge{i}_{c}")
                        nc.sync.dma_start(
                            kgt[:],
                            _ap(kout1_d, c * KH + i * 128 * TOK,
                                [[TOK, 128], [1, TOK]]))
                        kg_early[(i, c)] = kgt
                # ---- Phase D: local diagonal for ALL heads first. No
                # gather dependency: this fills the k/v AllGather window.
                # Each head's diagonal goes through a short PSUM chain and
                # is spilled to dacc (== ctx_acc) in SBUF.
                for i in range(6):
                    for half in range(2):
                        h = 2 * i + half
                        pv = pv_ps.tile([128, TOK], f32, name="pv")
                        d_eps = []
                        for hkt in range(2):
                            sT = st_ps.tile([128, 2, TOK], f32, name="sT")
                            ep = e_pool.tile([128, 2, TOK], bf16,
                                             name=f"epd{hkt}")
                            for sub in range(2):
                                kt = 2 * hkt + sub
                                q0 = kt * 128
                                nc.tensor.matmul(
                                    sT[:, sub, q0:TOK],
                                    k_pair[i][:, kt * 128:(kt + 1) * 128],
                                    qz[i][:, half, q0:TOK],
                                    start=True, stop=True)
                                nc.scalar.activation(
                                    ep[:, sub, q0:TOK], sT[:, sub, q0:TOK],
                                    AF.Exp, bias=0.0, scale=SCALE)
                                nc.gpsimd.tensor_mul(
                                    ep[:, sub, q0:q0 + 128],
                                    ep[:, sub, q0:q0 + 128], m01[:])
                            d_eps.append(ep)
                        for hkt in range(2):
                            for sub in range(2):
                                kt = 2 * hkt + sub
                                q0 = kt * 128
                                nc.tensor.matmul(
                                    pv[:, q0:TOK],
                                    v_t[kt][:, h * VROW:h * VROW + 128],
                                    d_eps[hkt][:, sub, q0:TOK],
                                    start=(kt == 0), stop=(kt == 3))
                        nc.vector.tensor_copy(ctx_acc[h][:], pv[0:VROW, :])

                # gathered v (fp8), loaded AFTER phase D so the DVE's
                # in-order queue is not poisoned by ops waiting on the v
                # gather; pad tail memset precedes the DMA so only the zm
                # scale waits on the collective
                va = []
                for c in range(NCH):
                    for kt in range(4):
                        vat = va_pool.tile([128, HROW + 63], f8,
                                           name=f"vat{c}_{kt}")
                        nc.vector.memset(vat[:, HROW:HROW + 63], 0.0)
                        nc.sync.dma_start(
                            vat[:, 0:HROW],
                            _ap(vout_d, c * VBLK + kt * 128 * HROW,
                                [[HROW, 128], [1, HROW]]))
                        nc.vector.tensor_scalar_mul(
                            vat[:, 0:HROW], vat[:, 0:HROW], zm[:, c:c + 1])
                        va.append(vat)

                # ---- Phase G: gathered chunks, software-pipelined so the
                # PV matmuls of group g-1 issue after the scores of group g
                # (exp latency hidden from the in-order PE queue).
                for i in range(6):
                    if i < 3:
                        kg = [kg_early[(i, c)] for c in range(NCH)]
                    else:
                        kg = []
                        for c in range(NCH):
                            kgt = kg_pool.tile([128, TOK], bf16,
                                               name=f"kg{c}")
                            nc.sync.dma_start(
                                kgt[:],
                                _ap(kout2_d, c * KH + (i - 3) * 128 * TOK,
                                    [[TOK, 128], [1, TOK]]))
                            kg.append(kgt)
                    # trickle hoisted weight DMAs behind this pair's loads
                    nc.sync.dma_start(
                        apw[i][:],
                        _ap(apw_d, i * 128 * C, [[C, 128], [1, C]]))
                    nc.sync.dma_start(
                        fcw[i][:],
                        _ap(fcw_d, i * 128 * FF, [[FF, 128], [1, FF]]))
                    for half in range(2):
                        h = 2 * i + half
                        pv = pv_ps.tile([128, TOK], f32, name="pv")
                        glist = [(c, hkt) for c in range(NCH)
                                 for hkt in range(2)]
                        g_eps = {}

                        def scores(g, i=i, half=half, kg=kg, g_eps=g_eps):
                            c, hkt = glist[g]
                            sT = st_ps.tile([128, 2, TOK], f32, name="sT")
                            ep = e_pool.tile([128, 2, TOK], bf16,
                                             name=f"ep{g % 3}")
                            for sub in range(2):
                                kt = 2 * hkt + sub
                                nc.tensor.matmul(
                                    sT[:, sub, :],
                                    kg[c][:, kt * 128:(kt + 1) * 128],
                                    qz[i][:, half, :],
                                    start=True, stop=True)
                            nc.scalar.activation(
                                ep[:, :, :], sT[:, :, :],
                                AF.Exp, bias=0.0, scale=SCALE)
                            g_eps[g] = ep

                        def pvs(g, h=h, pv=pv, g_eps=g_eps):
                            c, hkt = glist[g]
                            for sub in range(2):
                                kt = 2 * hkt + sub
                                nc.tensor.matmul(
                                    pv[:],
                                    va[c * 4 + kt][:,
                                                   h * VROW:h * VROW + 128],
                                    g_eps[g][:, sub, :],
                                    start=(g == 0 and sub == 0),
                                    stop=(g == len(glist) - 1 and sub == 1))

                        scores(0)
                        for g in range(1, len(glist)):
                            scores(g)
                            pvs(g - 1)
                        pvs(len(glist) - 1)
                        # combine with diagonal, then normalize inline
                        acc = ctx_acc[h]
                        nc.vector.tensor_add(acc[:], acc[:], pv[0:VROW, :])
                        rb = rb_ps.tile([64, TOK], f32, name="rb")
                        nc.tensor.matmul(rb[:], ones_b[64:65, 0:64],
                                         acc[64:65, :],
                                         start=True, stop=True)
                        rcp = nrm_pool.tile([64, TOK], f32, name="rcp")
                        with nc.allow_low_precision(reason="softmax denom"):
                            nc.vector.reciprocal(rcp[:], rb[:])
                        hb = 64 * half
                        nc.vector.tensor_mul(
                            ctx_pair[i][hb:hb + 64, :], acc[0:64, :],
                            rcp[:])

            # ------- attn_proj + residual, LN2 sums fused in -------
            x1T = [res_pool.tile([128, TOK], f32r, name=f"x1T{k}")
                   for k in range(KC)]
            h2 = [res_pool.tile([128, TOK], bf16, name=f"h2{k}")
                  for k in range(KC)]
            with (
                tc.tile_pool(name="apps", bufs=2, space="PSUM") as ap_ps,
                tc.tile_pool(name="lnps2", bufs=1, space="PSUM") as lps2,
                tc.tile_pool(name="lnab2", bufs=2, space="PSUM") as aps2,
                tc.tile_pool(name="lnsb2", bufs=3) as lsb2,
            ):
                sum_x = lps2.tile([1, TOK], f32, name="sumx2")
                sum_sq = lps2.tile([1, TOK], f32, name="sumsq2")
                for kc in range(KC):
                    ps = ap_ps.tile([128, TOK], f32, name="app")
                    for p in range(6):
                        nc.tensor.matmul(
                            ps[:], apw[p][:, kc * 128:(kc + 1) * 128],
                            ctx_pair[p][:], start=(p == 0), stop=(p == 5))
                    nc.vector.scalar_tensor_tensor(
                        out=x1T[kc][:], in0=ps[:], scalar=apb[:, kc:kc + 1],
                        in1=xT[kc][:], op0=OP.add, op1=OP.add)
                    sq = lsb2.tile([128, TOK], f32r, name="sq2")
                    nc.scalar.activation(sq[:], x1T[kc][:], AF.Square)
                    nc.tensor.matmul(sum_x[:], ones_r[:, 0:1], x1T[kc][:],
                                     start=(kc == 0), stop=(kc == KC - 1))
                    nc.tensor.matmul(sum_sq[:], ones_r[:, 0:1], sq[:],
                                     start=(kc == 0), stop=(kc == KC - 1))
                mu = lsb2.tile([1, TOK], f32, name="mu2")
                nc.scalar.activation(mu[:], sum_x[:], AF.Copy,
                                     bias=0.0, scale=1.0 / C)
                ex2 = lsb2.tile([1, TOK], f32, name="ex22")
                nc.scalar.activation(ex2[:], sum_sq[:], AF.Copy,
                                     bias=0.0, scale=1.0 / C)
                var = lsb2.tile([1, TOK], f32, name="var2")
                nc.vector.tensor_mul(var[:], mu[:], mu[:])
                nc.vector.tensor_sub(var[:], ex2[:], var[:])
                sd = lsb2.tile([1, TOK], f32, name="sd2")
                nc.scalar.activation(sd[:], var[:], AF.Sqrt,
                                     bias=eps_t[:], scale=1.0)
                rstd = lsb2.tile([1, TOK], bf16, name="rstd2")
                with nc.allow_low_precision(reason="bf16 rstd for PE"):
                    nc.vector.reciprocal(rstd[:], sd[:])
                nmr = lsb2.tile([1, TOK], bf16, name="nmr2")
                nc.vector.scalar_tensor_tensor(
                    out=nmr[:], in0=mu[:], scalar=-1.0, in1=rstd[:],
                    op0=OP.mult, op1=OP.mult)
                for k in range(KC):
                    a_ps = aps2.tile([128, TOK], f32, name="aps2")
                    b_ps = aps2.tile([128, TOK], f32, name="bps2")
                    sl = slice(k * 128, (k + 1) * 128)
                    nc.tensor.matmul(a_ps[:], sc2[0:1, sl], rstd[:],
                                     start=True, stop=True)
                    nc.tensor.matmul(b_ps[:], sc2[0:1, sl], nmr[:],
                                     start=True, stop=False)
                    nc.tensor.matmul(b_ps[:], sb2[0:1, sl],
                                     ones_row_b[:], start=False, stop=True)
                    tmp = lsb2.tile([128, TOK], f32, name="tmp2")
                    nc.vector.tensor_mul(tmp[:], x1T[k][:], a_ps[:])
                    nc.vector.tensor_add(h2[k][:], tmp[:], b_ps[:])

            qa_ctx.close()

            o_sb = [res_pool.tile([128, TOK], f32, name=f"o_sb{kc}")
                    for kc in range(KC)]
            with (
                tc.tile_pool(name="pjwp", bufs=1) as pjw_pool,
                tc.tile_pool(name="gsb", bufs=1) as g_pool,
                tc.tile_pool(name="otm", bufs=8) as otm_pool,
                tc.tile_pool(name="pops", bufs=2, space="PSUM") as po_ps,
            ):
                # phase A: all fc + gelu, keeping g[m] and pw[m] resident
                gs, pws = {}, {}
                fc_ctx = contextlib.ExitStack()
                fc_ps = fc_ctx.enter_context(
                    tc.tile_pool(name="fcps", bufs=4, space="PSUM"))
                for m in range(MT):
                    pw = pjw_pool.tile([128, C], bf16, name=f"pw{m}")
                    nc.sync.dma_start(
                        pw[:], _ap(pjw_d, m * 128 * C, [[C, 128], [1, C]]))
                    pws[m] = pw
                    gp = fc_ps.tile([128, TOK], f32, name="gp")
                    for k in range(KC):
                        nc.tensor.matmul(
                            gp[:], fcw[k][:, m * 128:(m + 1) * 128], h2[k][:],
                            start=(k == 0), stop=(k == KC - 1))
                    g = g_pool.tile([128, TOK], bf16, name=f"g{m}")
                    nc.scalar.activation(g[:], gp[:], AF.Gelu_apprx_tanh,
                                         bias=fcb[:, m:m + 1], scale=1.0)
                    gs[m] = g
                fc_ctx.close()
                # phase B: per output block, accumulate proj then finalize
                # (residual add, transpose to token-major, store) overlapped
                with tc.tile_pool(name="ops", bufs=2, space="PSUM") as o_ps:
                    for kc in range(KC):
                        pos = po_ps.tile([128, TOK], f32, name="po")
                        for m in range(MT):
                            nc.tensor.matmul(
                                pos[:],
                                pws[m][:, kc * 128:(kc + 1) * 128],
                                gs[m][:],
                                start=(m == 0), stop=(m == MT - 1))
                        nc.vector.scalar_tensor_tensor(
                            out=o_sb[kc][:], in0=pos[:],
                            scalar=pjb[:, kc:kc + 1],
                            in1=x1T[kc][:], op0=OP.add, op1=OP.add)
                        for tt in range(4):
                            tp2 = o_ps.tile([128, 128], f32, name="tp2")
                            nc.tensor.transpose(
                                tp2[:], o_sb[kc][:, tt * 128:(tt + 1) * 128],
                                ident[:])
                            ob = otm_pool.tile([128, 128], f32, name="ob")
                            nc.vector.tensor_copy(ob[:], tp2[:])
                            nc.sync.dma_start(
                                _ap(out_d, tt * 128 * C + kc * 128,
                                    [[C, 128], [1, 128]]),
                                ob[:])

    nc.compile()
    return nc


def kernel(x, mask, ln1_scale, ln1_bias, wqkv, bqkv, attn_proj_w, attn_proj_b,
           ln2_scale, ln2_bias, fc_w, fc_b, proj_w, proj_b):
    import ml_dtypes
    bf = ml_dtypes.bfloat16
    x = np.asarray(x, dtype=np.float32)
    if "nc" not in _CACHE:
        _CACHE["nc"] = _build()
    nc = _CACHE["nc"]

    shared = {
        "wqkv": np.ascontiguousarray(
            np.asarray(wqkv, np.float32).reshape(C, 3 * C).astype(bf)),
        "bqkv": np.ascontiguousarray(
            np.asarray(bqkv, np.float32).reshape(3 * C)),
        "bqv": np.ascontiguousarray(
            np.asarray(bqkv, np.float32).reshape(3 * C)[2 * C:].astype(bf)),
        "apw": np.ascontiguousarray(
            np.asarray(attn_proj_w, np.float32).astype(bf)),
        "apb": np.ascontiguousarray(np.asarray(attn_proj_b, np.float32)),
        "ln1s": np.ascontiguousarray(np.asarray(ln1_scale, np.float32).astype(bf)),
        "ln1b": np.ascontiguousarray(np.asarray(ln1_bias, np.float32).astype(bf)),
        "ln2s": np.ascontiguousarray(np.asarray(ln2_scale, np.float32).astype(bf)),
        "ln2b": np.ascontiguousarray(np.asarray(ln2_bias, np.float32).astype(bf)),
        "fcw": np.ascontiguousarray(np.asarray(fc_w, np.float32).astype(bf)),
        "fcb": np.ascontiguousarray(np.asarray(fc_b, np.float32)),
        "pjw": np.ascontiguousarray(np.asarray(proj_w, np.float32).astype(bf)),
        "pjb": np.ascontiguousarray(np.asarray(proj_b, np.float32)),
    }
    in_maps = []
    for core in range(N_CORES):
        b, r = divmod(core, GROUP)
        m = dict(shared)
        m["x"] = np.ascontiguousarray(x[b, r * TOK:(r + 1) * TOK, :])
        m["zmask"] = (np.arange(GROUP) < r).astype(np.float32)
        in_maps.append(m)

    res = run_bass_kernel_spmd(nc, in_maps, list(range(N_CORES)))
    _CACHE["last_result"] = res
    out = np.empty((B, T, C), dtype=np.float32)
    for core in range(N_CORES):
        b, r = divmod(core, GROUP)
        out[b, r * TOK:(r + 1) * TOK, :] = res.results[core]["out"]
    return out
